# revision 27
# baseline (speedup 1.0000x reference)
"""Distributed Trainium2 kernel for nn_CONNECT_86964497809993 (TGN-style
GNN message passing: last-event aggregation + GRU memory update + community
incidence matmul), sharded over 8 NeuronCores by node id.

Strategy: nodes are block-sharded across 8 cores (12500 each). Event routing
("last message per node") is integer index plumbing done on the host during
input sharding. Each core runs the FP pipeline ONLY for its nodes that
actually received a message (~63%, compacted to 8192 padded slots); the
remaining "inactive" nodes contribute just their (memory x incidence)
product to the community matmul.

All matmuls run in fp8(e4m3) DoubleRow mode (2 contraction blocks per pass,
0.5 cycles/col in the cost model). fp8 alone is too coarse: the fixed weight
quantization error rectifies through the GRU nonlinearities into a per-column
bias that the community reduction amplifies by sum(inc) ~ 3e4. So every
weight matrix is shipped as a RESIDUAL PAIR W ~ (W_hi + W_lo)/16 (both e4m3,
prescaled x16 to dodge subnormals; compensated via the activation-scale input
of sigmoid/tanh), making weight error second-order. The stream (per-node)
quantization noise stays first-order but zero-mean, which the community sum
averages away. The incidence matrix is quantized as q8(inc - 0.5) — halving
its absolute error — and the 0.5 * colsum(h') correction term is added back
on the host from the returned h' itself (exact, free).

Per-core device pipeline (64 active tiles of 128 nodes):
  gates  psum[r|z|xn] = DR(sd,Wsd_hi) + DR(sd,Wsd_lo) + DR(tenc,[Wt_hi|Wt_lo])
                        + DR(fm,Wfm_hi) + DR(fm,Wfm_lo) + DR(fm,W3a_hi/lo)
         psum[hn]     = DR(fm,W3b_hi) + DR(fm,W3b_lo)
         (tenc rides one DR pass via a stride-0 lhsT broadcast: block0 x hi
         rows, block1 x lo rows of the same 66-row tenc tile)
  GRU    r,z = sigmoid(psum/16) [ACT]; tt = r*hn; npre = xn + tt [DVE];
         n = tanh(npre/16) [ACT]; h' = n + z*(mem16 - n) [DVE f16 4x]
  comm   comm[M,C] += h'_tile.T @ inc_tile (f16 lhsT x fp8 rhs) per active
         tile; inactive tiles go as one DR pass each: (memI_hi, memI_lo)
         k-blocks x a stride-0 broadcast incI rhs.
Time encoding cos(dt*w+b) is computed on the host (index-gathered dt, same
two-step fp32 rounding as the reference) and shipped as fp8. All DRAM
operands are pre-tiled on the host so every DMA moves >=512B contiguous
chunks (full bandwidth); node-major tensors use a [128, tile, feat] layout
(node = tile*128 + partition). GRU/time params are replicated to all cores;
the [M,C] community partials are summed on the host (8 small tiles)."""

import numpy as np
import ml_dtypes

from concourse import bacc
import concourse.mybir as mybir
from concourse.tile import TileContext
from concourse.bass_utils import run_bass_kernel_spmd

# Problem shapes (hardcoded per contract).
N, E, C = 100000, 50000, 256
M, D, F, T = 128, 128, 128, 64
NCORES = 8
NPC = N // NCORES          # 12500 nodes per core
P = 128

ATILES = 64                # active slots: 64 tiles of 128 = 8192 (max count 7959)
NA = ATILES * P
ITILES = 38                # inactive slots: 38 tiles = 4864 (max count 4647)
SLABS = [4, 4, 16, 16, 16, 4, 2, 2]    # tiles per pipeline slab (sum = ATILES)
ISPLIT = [0, 0, 0, 10, 10, 10, 8, 0]   # inactive tiles interleaved after each slab
WSCALE = 16.0              # fp8 weight prescale; compensated in ACT scale
INV_WSCALE = 1.0 / WSCALE

f32 = mybir.dt.float32
f16 = mybir.dt.float16
f8 = mybir.dt.float8e4
E4 = ml_dtypes.float8_e4m3
A = mybir.AluOpType
AF = mybir.ActivationFunctionType
DR = mybir.MatmulPerfMode.DoubleRow

_COMPILED = None  # compiled Bacc program cache — build once per process


def _build_program():
    nc = bacc.Bacc("TRN2", target_bir_lowering=False)

    # Streams, feature-major [feat, 2, slot] fp8 (two DoubleRow k-blocks)
    sd8 = nc.dram_tensor("sd8", [P, 2, NA], f8, kind="ExternalInput")    # src_emb | dst_emb
    fm8 = nc.dram_tensor("fm8", [P, 2, NA], f8, kind="ExternalInput")    # feat | memory
    tc8 = nc.dram_tensor("tc8", [66, NA], f8, kind="ExternalInput")      # 64 freq rows + bias + pad
    # Node-major pre-tiled [partition, tile, ...]
    memA = nc.dram_tensor("memA", [P, ATILES, M], f16, kind="ExternalInput")
    incA = nc.dram_tensor("incA", [P, ATILES, C], f8, kind="ExternalInput")
    memI = nc.dram_tensor("memI", [P, ITILES, 2, M], f8, kind="ExternalInput")  # hi|lo residual pair
    incI = nc.dram_tensor("incI", [P, ITILES, C], f8, kind="ExternalInput")
    # Replicated weights (prescaled x16, fp8 hi/lo residual pairs), packed as
    # two blobs so startup is 2 DMA issues instead of 9:
    # Wblob [128, 2, 1536]: cols [0:384]=sd_hi, [384:768]=sd_lo, [768:1024]=fm_hi,
    # [1024:1280]=fm_lo (all k-block = stream pair), then [1280:1408]=Wc_n and
    # [1408:1536]=Whh_n with k-block = (hi, lo)
    Wblob = nc.dram_tensor("Wblob", [P, 2, 1536], f8, kind="ExternalInput")
    Wthl = nc.dram_tensor("Wthl", [66, 2, 3 * M], f8, kind="ExternalInput")  # block0=hi, block1=lo

    om = nc.dram_tensor("om", [P, ATILES, M], f16, kind="ExternalOutput")
    ocm = nc.dram_tensor("ocm", [M, C], f32, kind="ExternalOutput")

    n_comm = ATILES + ITILES  # active tiles + inactive single-pass DR tiles

    with TileContext(nc) as tc:
        with tc.tile_pool(name="const", bufs=1) as cpool, \
             tc.tile_pool(name="small", bufs=4) as spool, \
             tc.tile_pool(name="slab", bufs=3) as pool, \
             tc.tile_pool(name="ps", bufs=3, space="PSUM") as pspool, \
             tc.tile_pool(name="pacc", bufs=1, space="PSUM") as paccpool:

            with tc.high_priority():
                wb_t = cpool.tile([P, 2, 1536], f8, tag="wblob")
                nc.sync.dma_start(wb_t[:], Wblob[:])
                wthl_t = cpool.tile([66, 2, 3 * M], f8, tag="wthl")
                nc.sync.dma_start(wthl_t[:], Wthl[:])
            wsdh_t = wb_t[:, :, 0:384]
            wsdl_t = wb_t[:, :, 384:768]
            wfmh_t = wb_t[:, :, 768:1024]
            wfml_t = wb_t[:, :, 1024:1280]
            # 3a/3b tiles pair (hi, lo) as the two DR k-blocks (the stream
            # side broadcasts one fm block into both)
            w3a_t = wb_t[:, :, 1280:1408]
            w3b_t = wb_t[:, :, 1408:1536]
            memI_t = cpool.tile([P, ITILES, 2, M], f8)
            incI_t = cpool.tile([P, ITILES, C], f8)

            comm_acc = paccpool.tile([M, C], f32)

            state = {"comm": 0, "itile": 0}

            def comm_flags():
                i = state["comm"]
                state["comm"] += 1
                return dict(start=(i == 0), stop=(i == n_comm - 1))

            def emit_slab(t0, st, prev_finish):
                """Emit the DMA loads + gate pipeline for one slab. The tanh of
                each pair is skewed one pair behind its sigmoid so ACT never
                head-of-line blocks on the DVE tt/npre hop. Returns a finisher
                closure (blend + community + store), which the caller runs one
                slab later to keep PE/DVE from stalling on the blend chain."""
                g0 = t0 * P
                w = st * P
                import contextlib
                prio = tc.high_priority() if t0 == 0 else contextlib.nullcontext()
                with prio:
                    fm_s = pool.tile([P, 2, w], f8, tag="fm")
                    nc.sync.dma_start(fm_s[:], fm8[:, :, g0:g0 + w])
                    tc_s = pool.tile([66, w], f8, tag="tc")
                    nc.sync.dma_start(tc_s[:], tc8[:, g0:g0 + w])
                    sd_s = pool.tile([P, 2, w], f8, tag="sd")
                    nc.sync.dma_start(sd_s[:], sd8[:, :, g0:g0 + w])
                mem_s = pool.tile([P, st, M], f16, tag="mem")
                nc.scalar.dma_start(mem_s[:], memA[:, t0:t0 + st, :])
                inc_s = pool.tile([P, st, C], f8, tag="inc")
                nc.sync.dma_start(inc_s[:], incA[:, t0:t0 + st, :])

                rsz = pool.tile([P, st, 2, M], f16, tag="rsz")
                n_sl = pool.tile([P, st, M], f16, tag="n_sl")

                pending = None  # (npre, i0) awaiting tanh
                pfin = list(prev_finish) if prev_finish else []

                def flush_tanh():
                    nonlocal pending
                    if pending is not None:
                        npre_p, i0_p = pending
                        nc.scalar.activation(n_sl[:, i0_p:i0_p + 2, :], npre_p[:],
                                             AF.Tanh, scale=INV_WSCALE)
                        pending = None

                for i0 in range(0, st, 2):
                    pz = pspool.tile([P, 2, 4, M], f32, tag="pz")
                    for j in range(2):
                        sl = slice((i0 + j) * P, (i0 + j + 1) * P)
                        tcb = tc_s[:, sl].rearrange("p (o n) -> p o n", o=1) \
                                         .broadcast_to([66, 2, P])
                        fonly = fm_s[:, 0:1, sl].broadcast_to([P, 2, P])
                        monly = fm_s[:, 1:2, sl].broadcast_to([P, 2, P])
                        # hn region: one DR pass, (m, m) blocks x (hi, lo)
                        nc.tensor.matmul(pz[:, j, 3, :], monly, w3b_t,
                                         start=True, stop=True, perf_mode=DR)
                        # r|z|xn region
                        nc.tensor.matmul(pz[:, j, 0:3, :], sd_s[:, :, sl], wsdh_t,
                                         start=True, stop=False, perf_mode=DR)
                        nc.tensor.matmul(pz[:, j, 0:3, :], sd_s[:, :, sl], wsdl_t,
                                         start=False, stop=False, perf_mode=DR)
                        nc.tensor.matmul(pz[:, j, 0:3, :], tcb, wthl_t[:],
                                         start=False, stop=False, perf_mode=DR)
                        nc.tensor.matmul(pz[:, j, 0:2, :], fm_s[:, :, sl], wfmh_t,
                                         start=False, stop=False, perf_mode=DR)
                        nc.tensor.matmul(pz[:, j, 0:2, :], fm_s[:, :, sl], wfml_t,
                                         start=False, stop=False, perf_mode=DR)
                        nc.tensor.matmul(pz[:, j, 2, :], fonly, w3a_t,
                                         start=False, stop=True, perf_mode=DR)

                    # r,z = sigmoid(psum/16); rsz layout [p, tile, gate, m]
                    nc.scalar.activation(rsz[:, i0:i0 + 2, :, :], pz[:, :, 0:2, :],
                                         AF.Sigmoid, scale=INV_WSCALE)
                    flush_tanh()
                    tt = spool.tile([P, 2, M], f32, tag="tt")
                    nc.vector.tensor_tensor(tt[:], pz[:, :, 3, :],
                                            rsz[:, i0:i0 + 2, 0, :], A.mult)
                    npre = spool.tile([P, 2, M], f32, tag="npre")
                    nc.vector.tensor_tensor(npre[:], pz[:, :, 2, :], tt[:], A.add)
                    pending = (npre, i0)
                    pair_idx = i0 // 2
                    if pfin and (pair_idx <= 1 or pair_idx >= 3):
                        pfin.pop(0)()   # blend halves after pairs 0-1, comm after pair 3
                flush_tanh()
                while pfin:
                    pfin.pop(0)()

                box = {}

                def fin_blend(half):
                    # GRU blend h' = n + z*(mem - n) (DVE 2x), emitted in
                    # half-slab chunks so it never head-blocks tt/npre in the
                    # DVE queue.
                    if half == 0:
                        d_new = pool.tile([P, st, M], f16, tag="d_sl")
                        out_new = pool.tile([P, st, M], f16, tag="out_sl")
                        box["d"], box["out"] = d_new, out_new
                    h0 = (st // 2) * half
                    h1 = st if half else st // 2
                    sl = slice(h0, h1)
                    d_sl, out_sl = box["d"], box["out"]
                    nc.vector.tensor_tensor(
                        d_sl[:, sl, :], mem_s[:, sl, :], n_sl[:, sl, :], A.subtract)
                    nc.vector.tensor_tensor(
                        d_sl[:, sl, :], d_sl[:, sl, :], rsz[:, sl, 1, :], A.mult)
                    nc.vector.tensor_tensor(
                        out_sl[:, sl, :], n_sl[:, sl, :], d_sl[:, sl, :], A.add)

                def fin_comm():
                    out_sl = box["out"]

                    # Community partials: comm[M,C] += h'_tile.T @ inc_tile
                    for s in range(st):
                        nc.tensor.matmul(comm_acc[:], out_sl[:, s, :], inc_s[:, s, :],
                                         **comm_flags())
                    nc.gpsimd.dma_start(om[:, t0:t0 + st, :], out_sl[:])
                return (lambda: fin_blend(0)), (lambda: fin_blend(1)), fin_comm

            def emit_inactive(k):
                # one DR pass per tile: (memI_hi, memI_lo) k-blocks x same incI
                for _ in range(k):
                    i = state["itile"]
                    if i >= ITILES:
                        return
                    incb = incI_t[:, i, :].rearrange("p (o n) -> p o n", o=1) \
                                          .broadcast_to([P, 2, C])
                    nc.tensor.matmul(comm_acc[:], memI_t[:, i, :, :], incb,
                                     perf_mode=DR, **comm_flags())
                    state["itile"] += 1

            t0 = 0
            prev_finish = None
            for si, st in enumerate(SLABS):
                prev_finish = emit_slab(t0, st, prev_finish)
                if si == 2:
                    # inactive-side loads ride the in-order SP queue behind
                    # slabs 0-3's stream loads
                    nc.sync.dma_start(memI_t[:], memI[:])
                elif si == 3:
                    nc.sync.dma_start(incI_t[:], incI[:])
                emit_inactive(ISPLIT[si])
                t0 += st
            for f in prev_finish:
                f()

            cm = spool.tile([M, C], f32, tag="cm")
            nc.scalar.activation(cm[:], comm_acc[:], AF.Copy)
            nc.sync.dma_start(ocm[:], cm[:])

    nc.compile()
    return nc


def _get_program():
    global _COMPILED
    if _COMPILED is None:
        _COMPILED = _build_program()
    return _COMPILED


def _q8(x):
    return np.asarray(x, np.float32).astype(E4)


def _hilo(x):
    """x -> (hi, lo) e4m3 residual pair with x ~ hi + lo."""
    hi = _q8(x)
    lo = _q8(x - hi.astype(np.float32))
    return hi, lo


def _pack_tiles(rows, ntiles, np_dt):
    """[n, feat] row-major -> [128, ntiles, feat] with node = tile*128 + p."""
    nfeat = rows.shape[1]
    outp = np.zeros((ntiles * P, nfeat), np.float32)
    outp[:rows.shape[0]] = rows
    return np.ascontiguousarray(
        outp.reshape(ntiles, P, nfeat).transpose(1, 0, 2)).astype(np_dt)


def _pack_k2(blk0, blk1, nslots):
    """two [n, 128] blocks -> [128, 2, nslots] feature-major fp8."""
    out = np.zeros((P, 2, nslots), np.float32)
    out[:, 0, :blk0.shape[0]] = blk0.T
    out[:, 1, :blk1.shape[0]] = blk1.T
    return np.ascontiguousarray(out).astype(E4)


def kernel(src, dst, t, last_update, event_feat, src_embeds, dst_embeds,
           nodes_memory, incidence, w_time, b_time, W_ih, W_hh, b_ih, b_hh):
    src = np.asarray(src); dst = np.asarray(dst); t = np.asarray(t)
    last_update = np.asarray(last_update)
    event_feat = np.asarray(event_feat, np.float32)
    src_embeds = np.asarray(src_embeds, np.float32)
    dst_embeds = np.asarray(dst_embeds, np.float32)
    nodes_memory = np.asarray(nodes_memory, np.float32)
    incidence = np.asarray(incidence, np.float32)
    w_time = np.asarray(w_time, np.float32); b_time = np.asarray(b_time, np.float32)
    W_ih = np.asarray(W_ih, np.float32); W_hh = np.asarray(W_hh, np.float32)
    b_ih = np.asarray(b_ih, np.float32); b_hh = np.asarray(b_hh, np.float32)

    # ---- Host routing: 'last' aggregation = stable-sort scatter (index-only) ----
    src_all = np.concatenate([src, dst])
    t_all = np.concatenate([t, t])
    perm = np.argsort(t_all, kind="stable")
    win = np.zeros(N, np.int64)
    win[src_all[perm]] = perm          # ascending rank; last write = newest event
    has = np.bincount(src_all, minlength=N) > 0

    dt_ev = t_all - last_update[src_all]      # int32, per event
    dtw = dt_ev[win].astype(np.float32)       # [N]

    # Winner event rows (flipped copies share the original arrays)
    lt = win < E
    ge = ~lt
    w0 = np.where(lt, win, win - E)
    emb_s = np.empty((N, D), np.float32)
    emb_d = np.empty((N, D), np.float32)
    emb_s[lt] = src_embeds[w0[lt]]
    emb_s[ge] = dst_embeds[w0[ge]]
    emb_d[lt] = dst_embeds[w0[lt]]
    emb_d[ge] = src_embeds[w0[ge]]
    feat = event_feat[w0]

    # Replicated weights: x16 prescale, e4m3 hi/lo residual pairs
    def k2w(b0, b1, cols):
        out = np.empty((P, 2, cols), np.float32)
        out[:, 0] = b0
        out[:, 1] = b1
        return out

    Wa, Wb, Wc = W_ih[0:D], W_ih[D:2 * D], W_ih[2 * D:2 * D + F]
    sdh, sdl = _hilo(k2w(Wa * WSCALE, Wb * WSCALE, 3 * M))
    fmh, fml = _hilo(k2w(Wc[:, 0:2 * M] * WSCALE, W_hh[:, 0:2 * M] * WSCALE, 2 * M))
    ah, al = _hilo(Wc[:, 2 * M:3 * M] * WSCALE)
    bh, bl = _hilo(W_hh[:, 2 * M:3 * M] * WSCALE)
    Wt_rows = np.zeros((66, 3 * M), np.float32)
    Wt_rows[0:T] = W_ih[2 * D + F:] * WSCALE
    Wt_rows[T] = (b_ih + b_hh) * WSCALE
    th, tl = _hilo(Wt_rows)
    thl = np.empty((66, 2, 3 * M), E4)
    thl[:, 0] = th
    thl[:, 1] = tl
    wblob = np.empty((P, 2, 1536), E4)
    for arr, o0, o1 in ((sdh, 0, 384), (sdl, 384, 768), (fmh, 768, 1024),
                        (fml, 1024, 1280)):
        wblob[:, :, o0:o1] = arr
    wblob[:, 0, 1280:1408] = ah
    wblob[:, 1, 1280:1408] = al
    wblob[:, 0, 1408:1536] = bh
    wblob[:, 1, 1408:1536] = bl
    wmaps = dict(Wblob=wblob, Wthl=thl)

    in_maps = []
    core_meta = []
    for c in range(NCORES):
        lo_, hi_ = c * NPC, (c + 1) * NPC
        hs = has[lo_:hi_]
        act = np.nonzero(hs)[0] + lo_          # global ids, sorted
        ina = np.nonzero(~hs)[0] + lo_
        na, ni = len(act), len(ina)
        if na > NA or ni > ITILES * P:
            raise RuntimeError(f"core {c}: {na} active / {ni} inactive exceed slots")

        # Time encoding on host (fp32, two-step like the reference), -> fp8
        x = dtw[act][:, None] * w_time[None, :] + b_time[None, :]
        tenc = np.cos(x)                       # [na, T]
        tc_rows = np.zeros((66, NA), np.float32)
        tc_rows[0:T, :na] = tenc.T
        tc_rows[T, :na] = 1.0                  # bias lane

        memI_f = np.zeros((ITILES * P, M), np.float32)
        memI_f[:ni] = nodes_memory[ina]
        ihi, ilo = _hilo(memI_f)
        memI_hl = np.empty((ITILES * P, 2, M), np.float32)
        memI_hl[:, 0] = ihi.astype(np.float32)
        memI_hl[:, 1] = ilo.astype(np.float32)
        memI_p = np.ascontiguousarray(
            memI_hl.reshape(ITILES, P, 2, M).transpose(1, 0, 2, 3)).astype(E4)
        # colsum of the exact values the device used as comm lhsT rows
        colsum_I = (ihi.astype(np.float32) + ilo.astype(np.float32)) \
            .sum(axis=0, dtype=np.float64)

        in_maps.append(dict(
            sd8=_pack_k2(emb_s[act], emb_d[act], NA),
            fm8=_pack_k2(feat[act], nodes_memory[act], NA),
            tc8=tc_rows.astype(E4),
            memA=_pack_tiles(nodes_memory[act], ATILES, np.float16),
            incA=_pack_tiles(incidence[act] - 0.5, ATILES, E4),
            memI=memI_p,
            incI=_pack_tiles(incidence[ina] - 0.5, ITILES, E4),
            **wmaps,
        ))
        core_meta.append((act, na, colsum_I))

    nc = _get_program()
    res = run_bass_kernel_spmd(nc, in_maps, core_ids=list(range(NCORES)))

    out = np.empty((N + C, M), np.float32)
    out[:N] = nodes_memory
    comm = np.zeros((M, C), np.float64)
    for c in range(NCORES):
        act, na, colsum_I = core_meta[c]
        omc = np.asarray(res.results[c]["om"])           # [128, ATILES, M] f16
        h = omc.transpose(1, 0, 2).reshape(NA, M)[:na].astype(np.float32)
        out[act] = h
        # add back the 0.5*colsum term from the inc offset quantization
        colsum = h.sum(axis=0, dtype=np.float64) + colsum_I
        comm += res.results[c]["ocm"] + 0.5 * colsum[:, None]
    out[N:] = comm.T.astype(np.float32)
    return out
